# revision 37
# baseline (speedup 1.0000x reference)
"""GPT2 (L=12, D=1024, H=16, S=512, B=4, V=16386) on 8 trn2 NeuronCores.

Scheme: token-data-parallel. Each core owns 256 tokens (2 causal-balanced
blocks of 128 within one batch; pair cores 2c/2c+1 split batch c).
Per layer: LN1 (DVE tree-sums off the f32 residual + reciprocal_approx_fast,
stats start while the previous MLP is still draining) -> pair-AllGather of
x_ln (bf16, hidden behind local q compute + a q-recompute HAM warm-keeper)
-> k/v for the whole batch from the gathered x_ln -> attention for own q
tokens over a uniform causal superset (the all-dead q0-half of chunks 1,3
is skipped in mask/exp/av; per-core additive masks keep the SPMD program
uniform) -> per-head softmax with additive pre-exp masking, exp from SBUF,
fast-approx reciprocal denominators -> proj/LN2/FC/gelu/MLP (MLP kk-outer,
2 passes, streamed weights). lm_head token-sharded, bf16 logit stores.

Layout: activations feature-major [D, tok], v token-major with a per-head
ones column so the av matmul also yields softmax denominators; LN affine
folded into weights host-side; weights bf16, PSUM + residual stream f32.
"""

import os
import numpy as np
import ml_dtypes

# ---- static config (must match reference.py) ----
L = 12
D = 1024
H = 16
DH = 64
S = 512
B = 4
V = 16386
EPS = 1e-5
SCALE = 1.0 / 8.0  # 1/sqrt(DH)

P = 128
KT = D // P           # 8 k-tiles over D
TOK = 256             # own tokens per core
SB = 512              # batch tokens (kv length)
FF = 4096
FFT = FF // P         # 32
VPAD = 16896          # 33 * 512
NV = VPAD // 512      # 33

BF = ml_dtypes.bfloat16

# rank-order kv column blocks: chunk cc -> seq block id
BB = [0, 3, 1, 2]
# core parity -> owned q blocks
QBLOCKS = {0: (0, 3), 1: (1, 2)}

N_LAYERS = int(os.environ.get("GPT2_N_LAYERS", str(L)))


def _build(n_layers):
    from concourse import bacc, bass, mybir
    import concourse.tile as tile

    F32 = mybir.dt.float32
    BD = mybir.dt.bfloat16
    AF = mybir.ActivationFunctionType
    OP = mybir.AluOpType

    nc = bacc.Bacc("TRN2", target_bir_lowering=False, debug=False,
                   num_devices=8)

    # ---- kernel I/O ----
    h0T = nc.dram_tensor("h0T", [D, TOK], F32, kind="ExternalInput").ap()
    wqkv = nc.dram_tensor("wqkv", [n_layers * D, 3 * D], BD,
                          kind="ExternalInput").ap()
    wproj = nc.dram_tensor("wproj", [n_layers * D, D], BD,
                           kind="ExternalInput").ap()
    wfc = nc.dram_tensor("wfc", [n_layers * D, FF], BD,
                         kind="ExternalInput").ap()
    wmlp = nc.dram_tensor("wmlp", [n_layers * FF, D], BD,
                          kind="ExternalInput").ap()
    whead = nc.dram_tensor("whead", [NV * KT * P, 512], BD,
                           kind="ExternalInput").ap()
    # biases per layer, laid out [n_layers*128, 72]:
    #   cols 0:8 q (8 ptiles), 8:16 k, 24:32 proj, 32:64 fc, 64:72 mlp
    bvec = nc.dram_tensor("bvec", [n_layers * P, 72], F32,
                          kind="ExternalInput").ap()
    # v bias as a row (applied via K=1 matmul): [n_layers, 1024] bf16
    bvrow = nc.dram_tensor("bvrow", [n_layers, D], BD,
                           kind="ExternalInput").ap()
    # causal masks, per cc: even cc [m|m] (512), odd cc [m_q1|m_q1] (256)
    masks = nc.dram_tensor("masks", [P, 4 * SB], BD,
                           kind="ExternalInput").ap()
    out = nc.dram_tensor("out", [TOK, VPAD], BD, kind="ExternalOutput").ap()

    # internal DRAM for the per-layer pair all-gather of k/v
    agin = []
    agout = []
    for l in range(n_layers):
        agin.append(nc.dram_tensor(f"agin{l}", [P, KT * TOK], BD,
                                   kind="Internal").ap())
        agout.append(nc.dram_tensor(f"agout{l}", [2 * P, KT * TOK], BD,
                                    kind="Internal").ap())

    from contextlib import ExitStack

    with tile.TileContext(nc) as tc:
        with ExitStack() as ctx:
            consts = ctx.enter_context(tc.tile_pool(name="consts", bufs=1))
            resid = ctx.enter_context(tc.tile_pool(name="resid", bufs=1))
            wpool = ctx.enter_context(tc.tile_pool(name="w", bufs=11))
            xlnp = ctx.enter_context(tc.tile_pool(name="xln", bufs=1))
            konp = ctx.enter_context(tc.tile_pool(name="kon", bufs=1))
            qtp = ctx.enter_context(tc.tile_pool(name="qt", bufs=1))
            ktp = ctx.enter_context(tc.tile_pool(name="kt", bufs=1))
            vfp = ctx.enter_context(tc.tile_pool(name="vf", bufs=1))
            erp = ctx.enter_context(tc.tile_pool(name="er", bufs=6))
            scbp = ctx.enter_context(tc.tile_pool(name="scb", bufs=4))
            ounp = ctx.enter_context(tc.tile_pool(name="oun", bufs=1))
            invp = ctx.enter_context(tc.tile_pool(name="inv", bufs=2))
            smp = ctx.enter_context(tc.tile_pool(name="sm", bufs=5))
            smbp = ctx.enter_context(tc.tile_pool(name="smb", bufs=2))
            vrp = ctx.enter_context(tc.tile_pool(name="vr", bufs=1))
            rfp = ctx.enter_context(tc.tile_pool(name="rf", bufs=2))
            accp = ctx.enter_context(tc.tile_pool(name="acc", bufs=4))
            lntp = ctx.enter_context(tc.tile_pool(name="lnt", bufs=2))
            gsb = ctx.enter_context(tc.tile_pool(name="gsb", bufs=1))
            obp = ctx.enter_context(tc.tile_pool(name="ob", bufs=2))
            pmm = ctx.enter_context(
                tc.tile_pool(name="pmm", bufs=2, space="PSUM"))
            pq4 = ctx.enter_context(
                tc.tile_pool(name="pq4", bufs=4, space="PSUM"))
            pavp = ctx.enter_context(
                tc.tile_pool(name="pav", bufs=2, space="PSUM"))

            # ---- constants ----
            ones_col = consts.tile([P, 1], BD)
            nc.vector.memset(ones_col, 1.0)
            ones_row = consts.tile([1, P], BD)
            nc.vector.memset(ones_row, 1.0)
            eps_sb = consts.tile([1, 1], F32)
            nc.vector.memset(eps_sb, EPS)
            scale_col = consts.tile([P, 1], F32)
            nc.vector.memset(scale_col, SCALE)
            mask_sb = consts.tile([P, 4, SB], BD)
            nc.sync.dma_start(
                out=mask_sb, in_=masks.rearrange("p (c t) -> p c t", c=4))
            ball = consts.tile([P, n_layers, 72], F32)
            nc.sync.dma_start(
                out=ball,
                in_=bvec.rearrange("(l p) j -> p l j", l=n_layers))

            # residual stream (feature-major, f32), ping-pong
            h_a = resid.tile([P, KT, TOK], F32)
            h_b = resid.tile([P, KT, TOK], F32)
            for kk in range(KT):
                nc.sync.dma_start(
                    out=h_a[:, kk, :],
                    in_=h0T[kk * P:(kk + 1) * P, :])

            # v gathered, token-major with per-head ones column (65th);
            # the ones columns are initialized once and survive all layers.
            vf = vfp.tile([P, 4, 1040], BD)
            nc.vector.memset(vf, 1.0)

            _ln_ctr = [0]

            def emit_ln(h_t, x_ln):
                """h_t: [P, KT, TOK] f32 residual -> x_ln bf16 normalized.

                Stats via DVE tree sums (start as h m-tiles land), one
                Rsqrt, broadcast by K=1 matmuls, 2-op apply per k-tile.
                """
                _ln_ctr[0] += 1
                li = _ln_ctr[0]

                # tree sum over the 8 feature tiles, peak 3 live tiles
                def tree(pairf, tag):
                    a = accp.tile([P, TOK], F32, tag=tag, name=f"a{tag}{li}")
                    b = accp.tile([P, TOK], F32, tag=tag, name=f"b{tag}{li}")
                    pairf(a, 0)
                    pairf(b, 1)
                    ab = accp.tile([P, TOK], F32, tag=tag,
                                   name=f"ab{tag}{li}")
                    nc.vector.tensor_add(ab, a, b)
                    c = accp.tile([P, TOK], F32, tag=tag, name=f"c{tag}{li}")
                    d = accp.tile([P, TOK], F32, tag=tag, name=f"d{tag}{li}")
                    pairf(c, 2)
                    pairf(d, 3)
                    cd = accp.tile([P, TOK], F32, tag=tag,
                                   name=f"cd{tag}{li}")
                    nc.vector.tensor_add(cd, c, d)
                    r = accp.tile([P, TOK], F32, tag=tag, name=f"r{tag}{li}")
                    nc.vector.tensor_add(r, ab, cd)
                    return r

                def sum_pair(dst, i):
                    nc.vector.tensor_add(dst, h_t[:, 2 * i, :],
                                         h_t[:, 2 * i + 1, :])

                def sq_pair(dst, i):
                    t0 = lntp.tile([P, TOK], F32, tag="sqt",
                                   name=f"t0_{li}_{i}")
                    t1 = lntp.tile([P, TOK], F32, tag="sqt",
                                   name=f"t1_{li}_{i}")
                    nc.vector.tensor_mul(t0, h_t[:, 2 * i, :],
                                         h_t[:, 2 * i, :])
                    nc.vector.tensor_mul(t1, h_t[:, 2 * i + 1, :],
                                         h_t[:, 2 * i + 1, :])
                    nc.vector.tensor_add(dst, t0, t1)

                acc = tree(sum_pair, "acc")
                acc2 = tree(sq_pair, "sq")
                accb = lntp.tile([P, TOK], BD, tag="accb")
                acc2b = lntp.tile([P, TOK], BD, tag="accb")
                nc.vector.tensor_copy(out=accb, in_=acc)
                nc.vector.tensor_copy(out=acc2b, in_=acc2)
                pstat = pavp.tile([1, 2 * TOK], F32, tag="pav")
                nc.tensor.matmul(pstat[:, 0:TOK], ones_col, accb,
                                 start=True, stop=True)
                nc.tensor.matmul(pstat[:, TOK:2 * TOK], ones_col, acc2b,
                                 start=True, stop=True)
                mu = smp.tile([1, TOK], F32, tag="sm")
                e2 = smp.tile([1, TOK], F32, tag="sm")
                m2 = smp.tile([1, TOK], F32, tag="sm")
                var = smp.tile([1, TOK], F32, tag="sm")
                a_f = smp.tile([1, TOK], F32, tag="sm")
                c_f = smp.tile([1, TOK], F32, tag="sm")
                nc.vector.tensor_scalar_mul(mu, pstat[:, 0:TOK], 1.0 / D)
                nc.vector.tensor_scalar_mul(e2, pstat[:, TOK:2 * TOK],
                                            1.0 / D)
                sd = smp.tile([1, TOK], F32, tag="sm")
                nc.vector.tensor_mul(m2, mu, mu)
                nc.vector.tensor_sub(var, e2, m2)
                nc.scalar.activation(sd, var, AF.Sqrt, bias=eps_sb)
                nc.vector.reciprocal_approx_fast(out=a_f, in_=sd)
                nc.vector.tensor_mul(c_f, mu, a_f)
                ab = smbp.tile([1, TOK], BD, tag="smb")
                cb = smbp.tile([1, TOK], BD, tag="smb")
                nc.vector.tensor_copy(out=ab, in_=a_f)
                nc.vector.tensor_copy(out=cb, in_=c_f)
                pA = pavp.tile([P, TOK], F32, tag="pav")
                pC = pavp.tile([P, TOK], F32, tag="pav")
                nc.tensor.matmul(pA, ones_row, ab, start=True, stop=True)
                nc.tensor.matmul(pC, ones_row, cb, start=True, stop=True)
                A_sb = lntp.tile([P, TOK], F32, tag="Asb")
                C_sb = lntp.tile([P, TOK], BD, tag="Csb")
                nc.vector.tensor_copy(out=A_sb, in_=pA)
                nc.vector.tensor_copy(out=C_sb, in_=pC)
                for kk in range(KT):
                    t = lntp.tile([P, TOK], BD, tag="lnt")
                    nc.vector.tensor_mul(t, h_t[:, kk, :], A_sb)
                    nc.vector.tensor_sub(x_ln[:, kk, :], t, C_sb)

            for l in range(n_layers):
                lb = ball[:, l, :]
                # ---- qkv weights (start DMA early) ----
                wq = [wpool.tile([P, 3 * D], BD, tag="w", name=f"wq{l}_{i}")
                      for i in range(KT)]
                for kk in range(KT):
                    nc.sync.dma_start(
                        out=wq[kk],
                        in_=wqkv[l * D + kk * P:l * D + (kk + 1) * P, :])
                vrow = vrp.tile([1, D], BD, tag="vrow")
                nc.sync.dma_start(out=vrow, in_=bvrow[l:l + 1, :])

                # ---- LN1 on own tokens ----
                x_ln = xlnp.tile([P, KT, TOK], BD, tag="xln")
                emit_ln(h_a, x_ln)

                # ---- all-gather x_ln across the pair ----
                nc.sync.dma_start(
                    out=agin[l].rearrange("p (k t) -> p k t", k=KT),
                    in_=x_ln)
                nc.gpsimd.collective_compute(
                    "AllGather", mybir.AluOpType.bypass,
                    replica_groups=[[0, 1], [2, 3], [4, 5], [6, 7]],
                    ins=[agin[l]],
                    outs=[agout[l]],
                )

                # ---- q from own x_ln (overlaps the collective) ----
                qT = qtp.tile([P, KT, TOK], BD)
                for m in range(KT):
                    pq_ = pmm.tile([P, SB], F32, tag="pmm")
                    for kk in range(KT):
                        nc.tensor.matmul(
                            pq_[:, :TOK],
                            wq[kk][:, m * P:(m + 1) * P],
                            x_ln[:, kk, :],
                            start=(kk == 0), stop=(kk == KT - 1))
                    nc.scalar.activation(qT[:, m, :], pq_[:, :TOK],
                                         AF.Identity,
                                         bias=lb[:, m:m + 1])

                # ---- HAM warm-keeper: recompute q while the gather is
                # in flight (identical values; keeps PE at full clock) ----
                for mw in range(KT + 2):
                    m = mw % KT
                    pq2 = pmm.tile([P, SB], F32, tag="pmm",
                                   name=f"pqw{l}_{mw}")
                    for kk in range(KT):
                        nc.tensor.matmul(
                            pq2[:, :TOK],
                            wq[kk][:, m * P:(m + 1) * P],
                            x_ln[:, kk, :],
                            start=(kk == 0), stop=(kk == KT - 1))
                    nc.scalar.activation(qT[:, m, :], pq2[:, :TOK],
                                         AF.Identity,
                                         bias=lb[:, m:m + 1])

                # ---- proj weights (prefetch during attention) ----
                wp = [wpool.tile([P, 4, D], BD, tag="w", name=f"wp{l}_{i}")
                      for i in range(2)]
                for g in range(2):
                    nc.sync.dma_start(
                        out=wp[g],
                        in_=wproj[l * D + g * 4 * P:l * D + (g + 1) * 4 * P, :]
                        .rearrange("(k p) c -> p k c", k=4))

                # ---- unpack gathered x_ln ----
                x_ag = ktp.tile([P, KT, SB], BD, tag="xag")
                for s in range(2):
                    nc.sync.dma_start(
                        out=x_ag[:, :, s * TOK:(s + 1) * TOK],
                        in_=agout[l][s * P:(s + 1) * P, :]
                        .rearrange("p (k t) -> p k t", k=KT))

                # ---- k for the whole batch: out [1024, SB] ----
                kTf = konp.tile([P, KT, SB], BD, tag="ktf")
                for m in range(KT):
                    pk = pmm.tile([P, SB], F32, tag="pmm")
                    for kk in range(KT):
                        nc.tensor.matmul(
                            pk,
                            wq[kk][:, D + m * P:D + (m + 1) * P],
                            x_ag[:, kk, :],
                            start=(kk == 0), stop=(kk == KT - 1))
                    nc.scalar.activation(kTf[:, m, :], pk,
                                         AF.Identity,
                                         bias=lb[:, 8 + m:9 + m])
                # ---- v (token-major into vf, ones col preserved) ----
                for tp in range(4):
                    for nn in range(2):
                        pv = pmm.tile([P, SB], F32, tag="pmm")
                        for kk in range(KT):
                            nc.tensor.matmul(
                                pv,
                                x_ag[:, kk, tp * P:(tp + 1) * P],
                                wq[kk][:, 2 * D + nn * 512:
                                       2 * D + (nn + 1) * 512],
                                start=(kk == 0), stop=False)
                        nc.tensor.matmul(
                            pv, ones_row,
                            vrow[:, nn * 512:(nn + 1) * 512],
                            start=False, stop=True)
                        nc.scalar.copy(
                            vf[:, tp, :].rearrange(
                                "p (g x) -> p g x", x=65)[:, nn * 8:(nn + 1) * 8, 0:64],
                            pv.rearrange("p (g x) -> p g x", x=64))

                # ---- attention: 8 head pairs, causal-superset chunks ----
                o_un = ounp.tile([P, KT, TOK], BD)

                for j in range(KT):
                    hA, hB = 2 * j, 2 * j + 1
                    # scores: separate PSUM tile per (cc, half)
                    sc = {}
                    for cc in (0, 2, 1, 3):
                        w = TOK if cc % 2 == 0 else P
                        for half, po in ((0, 0), (1, 64)):
                            t = pq4.tile([P, w], F32, tag="q4",
                                         name=f"sc{cc}_{half}_{l}_{j}")
                            sc[(cc, half)] = t
                            nc.tensor.matmul(
                                t,
                                kTf[po:po + 64, j,
                                    BBI[cc] * P:(BBI[cc] + 1) * P],
                                qT[po:po + 64, j,
                                   (0 if cc % 2 == 0 else P):TOK],
                                start=True, stop=True)
                    # scaled scores + additive mask (frees the PSUM
                    # bank fast), then exp from SBUF; er layout [A|B]
                    er = {}
                    for cc in (0, 2, 1, 3):
                        w = TOK if cc % 2 == 0 else P
                        scb = scbp.tile([P, 2 * w], BD, tag="scb",
                                        name=f"scb{cc}_{l}_{j}")
                        for half in (0, 1):
                            nc.vector.scalar_tensor_tensor(
                                out=scb[:, half * w:(half + 1) * w],
                                in0=sc[(cc, half)],
                                scalar=scale_col,
                                in1=mask_sb[:, cc,
                                            half * w:(half + 1) * w],
                                op0=OP.mult, op1=OP.add)
                        er[cc] = erp.tile([P, 2 * w], BD, tag="er",
                                          name=f"er{cc}_{l}_{j}")
                        nc.scalar.activation(er[cc], scb, AF.Exp)
                    # av: accumulate over chunks into [65, TOK] per head
                    for half, hh in ((0, hA), (1, hB)):
                        pav = pavp.tile([65, TOK], F32, tag="pav")
                        nc.tensor.matmul(
                            pav, vf[:, 0, hh * 65:(hh + 1) * 65],
                            er[0][:, half * TOK:(half + 1) * TOK],
                            start=True, stop=False)
                        nc.tensor.matmul(
                            pav, vf[:, 2, hh * 65:(hh + 1) * 65],
                            er[2][:, half * TOK:(half + 1) * TOK],
                            start=False, stop=False)
                        nc.tensor.matmul(
                            pav[:, P:TOK], vf[:, 1, hh * 65:(hh + 1) * 65],
                            er[1][:, half * P:(half + 1) * P],
                            start=False, stop=False)
                        nc.tensor.matmul(
                            pav[:, P:TOK], vf[:, 3, hh * 65:(hh + 1) * 65],
                            er[3][:, half * P:(half + 1) * P],
                            start=False, stop=True)
                        dsb = rfp.tile([1, TOK], F32, tag="dsb",
                                       name=f"dsb{l}_{j}_{half}")
                        nc.scalar.copy(dsb, pav[64:65, :])
                        o_tmp = invp.tile([64, TOK], BD, tag="otmp",
                                          name=f"otmp{l}_{j}_{half}")
                        nc.vector.tensor_copy(out=o_tmp, in_=pav[0:64, :])
                        rf_h = rfp.tile([1, TOK], F32, tag="rf",
                                        name=f"rf{l}_{j}_{half}")
                        nc.vector.reciprocal_approx_fast(
                            out=rf_h, in_=dsb)
                        rb_h = rfp.tile([1, TOK], BD, tag="rfb",
                                        name=f"rb{l}_{j}_{half}")
                        nc.vector.tensor_copy(out=rb_h, in_=rf_h)
                        pB = pavp.tile([64, TOK], F32, tag="pav",
                                       name=f"pB{l}_{j}_{half}")
                        nc.tensor.matmul(pB, ones_row[:, 0:64], rb_h,
                                         start=True, stop=True)
                        isb = invp.tile([64, TOK], BD, tag="isb",
                                        name=f"isb{l}_{j}_{half}")
                        nc.scalar.copy(isb, pB)
                        nc.vector.tensor_mul(
                            o_un[half * 64:(half + 1) * 64, j, :],
                            o_tmp, isb)

                wf = [wpool.tile([P, FF], BD, tag="w", name=f"wf{l}_{i}")
                      for i in range(KT)]
                for kk in range(KT):
                    nc.sync.dma_start(
                        out=wf[kk],
                        in_=wfc[l * D + kk * P:l * D + (kk + 1) * P, :])

                # ---- proj + residual: h_b = h_a + proj(o) + bias ----
                for m in range(KT):
                    pp = pmm.tile([P, SB], F32, tag="pmm")
                    for kk in range(KT):
                        nc.tensor.matmul(
                            pp[:, :TOK],
                            wp[kk // 4][:, kk % 4, m * P:(m + 1) * P],
                            o_un[:, kk, :],
                            start=(kk == 0), stop=(kk == KT - 1))
                    nc.vector.scalar_tensor_tensor(
                        out=h_b[:, m, :],
                        in0=pp[:, :TOK],
                        scalar=lb[:, 24 + m:25 + m],
                        in1=h_a[:, m, :],
                        op0=OP.add, op1=OP.add)

                # ---- LN2 + FC + gelu ----
                x_ln2 = xlnp.tile([P, KT, TOK], BD, tag="xln")
                emit_ln(h_b, x_ln2)
                g_sb = gsb.tile([P, FFT, TOK], BD)
                for m in range(FFT):
                    pf = pmm.tile([P, SB], F32, tag="pmm")
                    for kk in range(KT):
                        nc.tensor.matmul(
                            pf[:, :TOK],
                            wf[kk][:, m * P:(m + 1) * P],
                            x_ln2[:, kk, :],
                            start=(kk == 0), stop=(kk == KT - 1))
                    nc.scalar.activation(g_sb[:, m, :], pf[:, :TOK],
                                         AF.Gelu_apprx_tanh,
                                         bias=lb[:, 32 + m:33 + m])
                # ---- MLP (kk-outer, streamed weights) + residual ----
                wm = [wpool.tile([P, 4, D], BD, tag="w", name=f"wm{l}_{i}")
                      for i in range(8)]
                for g in range(8):
                    nc.sync.dma_start(
                        out=wm[g],
                        in_=wmlp[l * FF + g * 4 * P:l * FF + (g + 1) * 4 * P, :]
                        .rearrange("(k p) c -> p k c", k=4))
                for ph in range(2):
                    pml = [pq4.tile([P, TOK], F32, tag="q4",
                                    name=f"pml{l}_{ph}_{i}")
                           for i in range(4)]
                    for kk in range(FFT):
                        for mb in range(4):
                            m = 4 * ph + mb
                            nc.tensor.matmul(
                                pml[mb],
                                wm[kk // 4][:, kk % 4, m * P:(m + 1) * P],
                                g_sb[:, kk, :],
                                start=(kk == 0), stop=(kk == FFT - 1))
                    for mb in range(4):
                        m = 4 * ph + mb
                        nc.vector.scalar_tensor_tensor(
                            out=h_a[:, m, :],
                            in0=pml[mb],
                            scalar=lb[:, 64 + m:65 + m],
                            in1=h_b[:, m, :],
                            op0=OP.add, op1=OP.add)

            # ---- final LN + lm_head ----
            x_lnf = xlnp.tile([P, KT, TOK], BD, tag="xln")
            emit_ln(h_a, x_lnf)
            for nn in range(NV):
                wh = wpool.tile([P, KT, 512], BD, tag="w")
                nc.sync.dma_start(
                    out=wh,
                    in_=whead[nn * KT * P:(nn + 1) * KT * P, :]
                    .rearrange("(k p) c -> p k c", k=KT))
                for tp in range(2):
                    ph = pmm.tile([P, SB], F32, tag="pmm")
                    for kk in range(KT):
                        nc.tensor.matmul(
                            ph,
                            x_lnf[:, kk, tp * P:(tp + 1) * P],
                            wh[:, kk, :],
                            start=(kk == 0), stop=(kk == KT - 1))
                    ob = obp.tile([P, 512], BD)
                    nc.scalar.copy(ob, ph)
                    nc.sync.dma_start(
                        out=out[tp * P:(tp + 1) * P,
                                nn * 512:(nn + 1) * 512],
                        in_=ob)

    nc.compile()
    return nc


# chunk cc -> column block (x128) inside kTf's gathered token axis.
# Gathered order IS rank order == BB order, so chunk cc sits at block cc.
BBI = [0, 1, 2, 3]


_CACHE = {}


def _get_nc(n_layers):
    if n_layers not in _CACHE:
        _CACHE[n_layers] = _build(n_layers)
    return _CACHE[n_layers]


def _prep_host(inputs, n_layers):
    """Host-side: embeddings, LN-affine folding, layouts, per-core shards."""
    ids = np.asarray(inputs["input_ids"])
    tts = np.asarray(inputs["token_type_ids"])
    wte = np.asarray(inputs["wte"], np.float32)
    wtte = np.asarray(inputs["wtte"], np.float32)
    wpe = np.asarray(inputs["wpe"], np.float32)

    h0 = wte[ids] + wpe[None, :, :] + wtte[tts]          # [B, S, D]

    ln1_w = np.asarray(inputs["ln1_w"], np.float32)
    ln1_b = np.asarray(inputs["ln1_b"], np.float32)
    attn_w = np.asarray(inputs["attn_w"], np.float32)
    attn_b = np.asarray(inputs["attn_b"], np.float32)
    atp_w = np.asarray(inputs["atp_w"], np.float32)
    atp_b = np.asarray(inputs["atp_b"], np.float32)
    ln2_w = np.asarray(inputs["ln2_w"], np.float32)
    ln2_b = np.asarray(inputs["ln2_b"], np.float32)
    fc_w = np.asarray(inputs["fc_w"], np.float32)
    fc_b = np.asarray(inputs["fc_b"], np.float32)
    mlp_w = np.asarray(inputs["mlp_w"], np.float32)
    mlp_b = np.asarray(inputs["mlp_b"], np.float32)
    lnf_w = np.asarray(inputs["lnf_w"], np.float32)
    lnf_b = np.asarray(inputs["lnf_b"], np.float32)
    head_w = np.asarray(inputs["head_w"], np.float32)
    head_b = np.asarray(inputs["head_b"], np.float32)

    nl = n_layers
    wqkv = np.empty((nl * D, 3 * D), BF)
    wproj_ = np.empty((nl * D, D), BF)
    wfc_ = np.empty((nl * D, FF), BF)
    wmlp_ = np.empty((nl * FF, D), BF)
    bvec = np.zeros((nl * P, 72), np.float32)
    bvrow = np.zeros((nl, D), BF)
    for l in range(nl):
        wq = attn_w[l] * ln1_w[l][:, None]
        bq = attn_b[l] + ln1_b[l] @ attn_w[l]            # [3072]
        wqkv[l * D:(l + 1) * D] = wq.astype(BF)
        wproj_[l * D:(l + 1) * D] = atp_w[l].astype(BF)
        wfc_[l * D:(l + 1) * D] = (fc_w[l] * ln2_w[l][:, None]).astype(BF)
        wmlp_[l * FF:(l + 1) * FF] = mlp_w[l].astype(BF)
        bvec[l * P:(l + 1) * P, 0:8] = bq[0:D].reshape(8, P).T
        bvec[l * P:(l + 1) * P, 8:16] = bq[D:2 * D].reshape(8, P).T
        bvec[l * P:(l + 1) * P, 24:32] = atp_b[l].reshape(8, P).T
        bfc = fc_b[l] + ln2_b[l] @ fc_w[l]
        bvec[l * P:(l + 1) * P, 32:64] = bfc.reshape(32, P).T
        bvec[l * P:(l + 1) * P, 64:72] = mlp_b[l].reshape(8, P).T
        bvrow[l] = bq[2 * D:3 * D].astype(BF)            # v bias as row

    whf = (head_w * lnf_w[:, None]).astype(np.float32)
    whp = np.zeros((D, VPAD), np.float32)
    whp[:, :V] = whf
    whead = np.ascontiguousarray(
        whp.reshape(KT, P, NV, 512).transpose(2, 0, 1, 3)
    ).reshape(NV * KT * P, 512).astype(BF)
    bhost = lnf_b @ head_w + head_b                      # [V]


    in_maps = []
    for c in range(8):
        rho = c % 2
        batch = c // 2
        qb = QBLOCKS[rho]
        h0T = np.ascontiguousarray(
            np.concatenate(
                [h0[batch, qb[0] * P:(qb[0] + 1) * P],
                 h0[batch, qb[1] * P:(qb[1] + 1) * P]], axis=0).T
        ).astype(np.float32)                              # [D, TOK]
        # masks [P, 4*SB]: even cc [m|m], odd cc [m_q1|m_q1|zeros]
        mk = np.zeros((P, 4 * SB), BF)
        for cc in range(4):
            kb = BB[cc]
            m = np.zeros((P, TOK), np.float32)
            for qh in range(2):
                qblk = qb[qh]
                kg = kb * P + np.arange(P)[:, None]
                qg = qblk * P + np.arange(P)[None, :]
                m[:, qh * P:(qh + 1) * P] = \
                    np.where(kg <= qg, 0.0, -10000.0)
            if cc % 2 == 0:
                mk[:, cc * SB:cc * SB + TOK] = m.astype(BF)
                mk[:, cc * SB + TOK:(cc + 1) * SB] = m.astype(BF)
            else:
                mq1 = m[:, P:TOK]
                mk[:, cc * SB:cc * SB + P] = mq1.astype(BF)
                mk[:, cc * SB + P:cc * SB + TOK] = mq1.astype(BF)
        in_maps.append({
            "h0T": h0T,
            "wqkv": wqkv, "wproj": wproj_, "wfc": wfc_, "wmlp": wmlp_,
            "whead": whead, "bvec": bvec, "bvrow": bvrow,
            "masks": mk,
        })
    return in_maps, bhost


def kernel(**inputs):
    from concourse import bass_utils

    n_layers = N_LAYERS
    nc = _get_nc(n_layers)
    in_maps, bhost = _prep_host(inputs, n_layers)

    trace = bool(int(os.environ.get("GPT2_TRACE", "0")))
    res = bass_utils.run_bass_kernel_spmd(
        nc, in_maps, core_ids=list(range(8)), trace=trace)
    if trace:
        kernel.last_exec_time_ns = res.exec_time_ns
        kernel.last_results = res

    full = np.empty((B, S, V), np.float32)
    for c in range(8):
        o = res.results[c]["out"]                         # [TOK, VPAD]
        rho = c % 2
        batch = c // 2
        qb = QBLOCKS[rho]
        full[batch, qb[0] * P:(qb[0] + 1) * P] = o[0:P, :V]
        full[batch, qb[1] * P:(qb[1] + 1) * P] = o[P:2 * P, :V]
    full += bhost[None, None, :]
    return full


# revision 39
# speedup vs baseline: 1.0349x; 1.0349x over previous
"""GPT2 (L=12, D=1024, H=16, S=512, B=4, V=16386) on 8 trn2 NeuronCores.

Scheme: token-data-parallel. Each core owns 256 tokens (2 causal-balanced
blocks of 128 within one batch; pair cores 2c/2c+1 split batch c).
Per layer: LN1 (DVE tree-sums off the f32 residual + reciprocal_approx_fast,
stats start while the previous MLP is still draining) -> pair-AllGather of
x_ln (bf16, hidden behind local q compute + a q-recompute HAM warm-keeper)
-> k/v for the whole batch from the gathered x_ln -> attention for own q
tokens over a uniform causal superset (the all-dead q0-half of chunks 1,3
is skipped in mask/exp/av; per-core additive masks keep the SPMD program
uniform) -> per-head softmax with additive pre-exp masking, exp from SBUF,
fast-approx reciprocal denominators -> proj/LN2/FC/gelu/MLP (MLP kk-outer,
2 passes, streamed weights). lm_head token-sharded, bf16 logit stores.

Layout: activations feature-major [D, tok], v token-major with a per-head
ones column so the av matmul also yields softmax denominators; LN affine
folded into weights host-side; weights bf16, PSUM + residual stream f32.
"""

import os
import numpy as np
import ml_dtypes

# ---- static config (must match reference.py) ----
L = 12
D = 1024
H = 16
DH = 64
S = 512
B = 4
V = 16386
EPS = 1e-5
SCALE = 1.0 / 8.0  # 1/sqrt(DH)

P = 128
KT = D // P           # 8 k-tiles over D
TOK = 256             # own tokens per core
SB = 512              # batch tokens (kv length)
FF = 4096
FFT = FF // P         # 32
VPAD = 16896          # 33 * 512
NV = VPAD // 512      # 33

BF = ml_dtypes.bfloat16

# rank-order kv column blocks: chunk cc -> seq block id
BB = [0, 3, 1, 2]
# core parity -> owned q blocks
QBLOCKS = {0: (0, 3), 1: (1, 2)}

N_LAYERS = int(os.environ.get("GPT2_N_LAYERS", str(L)))


def _build(n_layers):
    from concourse import bacc, bass, mybir
    import concourse.tile as tile

    F32 = mybir.dt.float32
    BD = mybir.dt.bfloat16
    AF = mybir.ActivationFunctionType
    OP = mybir.AluOpType

    nc = bacc.Bacc("TRN2", target_bir_lowering=False, debug=False,
                   num_devices=8)

    # ---- kernel I/O ----
    h0T = nc.dram_tensor("h0T", [D, TOK], F32, kind="ExternalInput").ap()
    wqkv = nc.dram_tensor("wqkv", [n_layers * D, 3 * D], BD,
                          kind="ExternalInput").ap()
    wproj = nc.dram_tensor("wproj", [n_layers * D, D], BD,
                           kind="ExternalInput").ap()
    wfc = nc.dram_tensor("wfc", [n_layers * D, FF], BD,
                         kind="ExternalInput").ap()
    wmlp = nc.dram_tensor("wmlp", [n_layers * FF, D], BD,
                          kind="ExternalInput").ap()
    whead = nc.dram_tensor("whead", [NV * KT * P, 512], BD,
                           kind="ExternalInput").ap()
    # biases per layer, laid out [n_layers*128, 72]:
    #   cols 0:8 q (8 ptiles), 8:16 k, 24:32 proj, 32:64 fc, 64:72 mlp
    bvec = nc.dram_tensor("bvec", [n_layers * P, 72], F32,
                          kind="ExternalInput").ap()
    # v bias as a row (applied via K=1 matmul): [n_layers, 1024] bf16
    bvrow = nc.dram_tensor("bvrow", [n_layers, D], BD,
                           kind="ExternalInput").ap()
    # causal masks, per cc: even cc [m|m] (512), odd cc [m_q1|m_q1] (256)
    masks = nc.dram_tensor("masks", [P, 4 * SB], BD,
                           kind="ExternalInput").ap()
    out = nc.dram_tensor("out", [TOK, VPAD], BD, kind="ExternalOutput").ap()

    # internal DRAM for the per-layer pair all-gather of k/v
    agin = []
    agout = []
    for l in range(n_layers):
        agin.append(nc.dram_tensor(f"agin{l}", [P, KT * TOK], BD,
                                   kind="Internal").ap())
        agout.append(nc.dram_tensor(f"agout{l}", [2 * P, KT * TOK], BD,
                                    kind="Internal").ap())

    from contextlib import ExitStack

    with tile.TileContext(nc) as tc:
        with ExitStack() as ctx:
            consts = ctx.enter_context(tc.tile_pool(name="consts", bufs=1))
            resid = ctx.enter_context(tc.tile_pool(name="resid", bufs=1))
            wpool = ctx.enter_context(tc.tile_pool(name="w", bufs=11))
            xlnp = ctx.enter_context(tc.tile_pool(name="xln", bufs=1))
            konp = ctx.enter_context(tc.tile_pool(name="kon", bufs=1))
            qtp = ctx.enter_context(tc.tile_pool(name="qt", bufs=1))
            ktp = ctx.enter_context(tc.tile_pool(name="kt", bufs=1))
            vfp = ctx.enter_context(tc.tile_pool(name="vf", bufs=1))
            erp = ctx.enter_context(tc.tile_pool(name="er", bufs=4))
            scbp = ctx.enter_context(tc.tile_pool(name="scb", bufs=4))
            ounp = ctx.enter_context(tc.tile_pool(name="oun", bufs=1))
            invp = ctx.enter_context(tc.tile_pool(name="inv", bufs=2))
            smp = ctx.enter_context(tc.tile_pool(name="sm", bufs=5))
            smbp = ctx.enter_context(tc.tile_pool(name="smb", bufs=2))
            vrp = ctx.enter_context(tc.tile_pool(name="vr", bufs=1))
            rfp = ctx.enter_context(tc.tile_pool(name="rf", bufs=2))
            accp = ctx.enter_context(tc.tile_pool(name="acc", bufs=4))
            lntp = ctx.enter_context(tc.tile_pool(name="lnt", bufs=2))
            gsb = ctx.enter_context(tc.tile_pool(name="gsb", bufs=1))
            obp = ctx.enter_context(tc.tile_pool(name="ob", bufs=2))
            pmm = ctx.enter_context(
                tc.tile_pool(name="pmm", bufs=2, space="PSUM"))
            pq4 = ctx.enter_context(
                tc.tile_pool(name="pq4", bufs=4, space="PSUM"))
            pavp = ctx.enter_context(
                tc.tile_pool(name="pav", bufs=2, space="PSUM"))

            # ---- constants ----
            ones_col = consts.tile([P, 1], BD)
            nc.vector.memset(ones_col, 1.0)
            ones_row = consts.tile([1, P], BD)
            nc.vector.memset(ones_row, 1.0)
            eps_sb = consts.tile([1, 1], F32)
            nc.vector.memset(eps_sb, EPS)
            scale_col = consts.tile([P, 1], F32)
            nc.vector.memset(scale_col, SCALE)
            mask_sb = consts.tile([P, 4, SB], BD)
            nc.sync.dma_start(
                out=mask_sb, in_=masks.rearrange("p (c t) -> p c t", c=4))
            ball = consts.tile([P, n_layers, 72], F32)
            nc.sync.dma_start(
                out=ball,
                in_=bvec.rearrange("(l p) j -> p l j", l=n_layers))

            # residual stream (feature-major, f32), ping-pong
            h_a = resid.tile([P, KT, TOK], F32)
            h_b = resid.tile([P, KT, TOK], F32)
            for kk in range(KT):
                nc.sync.dma_start(
                    out=h_a[:, kk, :],
                    in_=h0T[kk * P:(kk + 1) * P, :])

            # v gathered, token-major with per-head ones column (65th);
            # the ones columns are initialized once and survive all layers.
            vf = vfp.tile([P, 4, 1040], BD)
            nc.vector.memset(vf, 1.0)

            _ln_ctr = [0]

            def emit_ln(h_t, x_ln):
                """h_t: [P, KT, TOK] f32 residual -> x_ln bf16 normalized.

                Stats via DVE tree sums (start as h m-tiles land), one
                Rsqrt, broadcast by K=1 matmuls, 2-op apply per k-tile.
                """
                _ln_ctr[0] += 1
                li = _ln_ctr[0]

                # tree sum over the 8 feature tiles, peak 3 live tiles
                def tree(pairf, tag):
                    a = accp.tile([P, TOK], F32, tag=tag, name=f"a{tag}{li}")
                    b = accp.tile([P, TOK], F32, tag=tag, name=f"b{tag}{li}")
                    pairf(a, 0)
                    pairf(b, 1)
                    ab = accp.tile([P, TOK], F32, tag=tag,
                                   name=f"ab{tag}{li}")
                    nc.vector.tensor_add(ab, a, b)
                    c = accp.tile([P, TOK], F32, tag=tag, name=f"c{tag}{li}")
                    d = accp.tile([P, TOK], F32, tag=tag, name=f"d{tag}{li}")
                    pairf(c, 2)
                    pairf(d, 3)
                    cd = accp.tile([P, TOK], F32, tag=tag,
                                   name=f"cd{tag}{li}")
                    nc.vector.tensor_add(cd, c, d)
                    r = accp.tile([P, TOK], F32, tag=tag, name=f"r{tag}{li}")
                    nc.vector.tensor_add(r, ab, cd)
                    return r

                def sum_pair(dst, i):
                    nc.vector.tensor_add(dst, h_t[:, 2 * i, :],
                                         h_t[:, 2 * i + 1, :])

                def sq_pair(dst, i):
                    t0 = lntp.tile([P, TOK], F32, tag="sqt",
                                   name=f"t0_{li}_{i}")
                    t1 = lntp.tile([P, TOK], F32, tag="sqt",
                                   name=f"t1_{li}_{i}")
                    nc.vector.tensor_mul(t0, h_t[:, 2 * i, :],
                                         h_t[:, 2 * i, :])
                    nc.vector.tensor_mul(t1, h_t[:, 2 * i + 1, :],
                                         h_t[:, 2 * i + 1, :])
                    nc.vector.tensor_add(dst, t0, t1)

                acc = tree(sum_pair, "acc")
                acc2 = tree(sq_pair, "sq")
                accb = lntp.tile([P, TOK], BD, tag="accb")
                acc2b = lntp.tile([P, TOK], BD, tag="accb")
                nc.vector.tensor_copy(out=accb, in_=acc)
                nc.vector.tensor_copy(out=acc2b, in_=acc2)
                pstat = pavp.tile([1, 2 * TOK], F32, tag="pav")
                nc.tensor.matmul(pstat[:, 0:TOK], ones_col, accb,
                                 start=True, stop=True)
                nc.tensor.matmul(pstat[:, TOK:2 * TOK], ones_col, acc2b,
                                 start=True, stop=True)
                mu = smp.tile([1, TOK], F32, tag="sm")
                e2 = smp.tile([1, TOK], F32, tag="sm")
                m2 = smp.tile([1, TOK], F32, tag="sm")
                var = smp.tile([1, TOK], F32, tag="sm")
                a_f = smp.tile([1, TOK], F32, tag="sm")
                c_f = smp.tile([1, TOK], F32, tag="sm")
                nc.vector.tensor_scalar_mul(mu, pstat[:, 0:TOK], 1.0 / D)
                nc.vector.tensor_scalar_mul(e2, pstat[:, TOK:2 * TOK],
                                            1.0 / D)
                sd = smp.tile([1, TOK], F32, tag="sm")
                nc.vector.tensor_mul(m2, mu, mu)
                nc.vector.tensor_sub(var, e2, m2)
                nc.scalar.activation(sd, var, AF.Sqrt, bias=eps_sb)
                nc.vector.reciprocal_approx_fast(out=a_f, in_=sd)
                nc.vector.tensor_mul(c_f, mu, a_f)
                ab = smbp.tile([1, TOK], BD, tag="smb")
                cb = smbp.tile([1, TOK], BD, tag="smb")
                nc.vector.tensor_copy(out=ab, in_=a_f)
                nc.vector.tensor_copy(out=cb, in_=c_f)
                pA = pavp.tile([P, TOK], F32, tag="pav")
                pC = pavp.tile([P, TOK], F32, tag="pav")
                nc.tensor.matmul(pA, ones_row, ab, start=True, stop=True)
                nc.tensor.matmul(pC, ones_row, cb, start=True, stop=True)
                A_sb = lntp.tile([P, TOK], F32, tag="Asb")
                C_sb = lntp.tile([P, TOK], BD, tag="Csb")
                nc.vector.tensor_copy(out=A_sb, in_=pA)
                nc.vector.tensor_copy(out=C_sb, in_=pC)
                for kk in range(KT):
                    t = lntp.tile([P, TOK], BD, tag="lnt")
                    nc.vector.tensor_mul(t, h_t[:, kk, :], A_sb)
                    nc.vector.tensor_sub(x_ln[:, kk, :], t, C_sb)

            for l in range(n_layers):
                lb = ball[:, l, :]
                # ---- qkv weights (start DMA early) ----
                wq = [wpool.tile([P, 3 * D], BD, tag="w", name=f"wq{l}_{i}")
                      for i in range(KT)]
                for kk in range(KT):
                    nc.sync.dma_start(
                        out=wq[kk],
                        in_=wqkv[l * D + kk * P:l * D + (kk + 1) * P, :])
                vrow = vrp.tile([1, D], BD, tag="vrow")
                nc.sync.dma_start(out=vrow, in_=bvrow[l:l + 1, :])

                # ---- LN1 on own tokens ----
                x_ln = xlnp.tile([P, KT, TOK], BD, tag="xln")
                emit_ln(h_a, x_ln)

                # ---- all-gather x_ln across the pair ----
                nc.sync.dma_start(
                    out=agin[l].rearrange("p (k t) -> p k t", k=KT),
                    in_=x_ln)
                nc.gpsimd.collective_compute(
                    "AllGather", mybir.AluOpType.bypass,
                    replica_groups=[[0, 1], [2, 3], [4, 5], [6, 7]],
                    ins=[agin[l]],
                    outs=[agout[l]],
                )

                # ---- q from own x_ln (overlaps the collective) ----
                qT = qtp.tile([P, KT, TOK], BD)
                for m in range(KT):
                    pq_ = pmm.tile([P, SB], F32, tag="pmm")
                    for kk in range(KT):
                        nc.tensor.matmul(
                            pq_[:, :TOK],
                            wq[kk][:, m * P:(m + 1) * P],
                            x_ln[:, kk, :],
                            start=(kk == 0), stop=(kk == KT - 1))
                    nc.scalar.activation(qT[:, m, :], pq_[:, :TOK],
                                         AF.Identity,
                                         bias=lb[:, m:m + 1])

                # ---- HAM warm-keeper: recompute q while the gather is
                # in flight (identical values; keeps PE at full clock) ----
                for m in range(KT):
                    pq2 = pmm.tile([P, SB], F32, tag="pmm",
                                   name=f"pqw{l}_{m}")
                    for kk in range(KT):
                        nc.tensor.matmul(
                            pq2[:, :TOK],
                            wq[kk][:, m * P:(m + 1) * P],
                            x_ln[:, kk, :],
                            start=(kk == 0), stop=(kk == KT - 1))
                    nc.scalar.activation(qT[:, m, :], pq2[:, :TOK],
                                         AF.Identity,
                                         bias=lb[:, m:m + 1])

                # ---- proj weights (prefetch during attention) ----
                wp = [wpool.tile([P, 4, D], BD, tag="w", name=f"wp{l}_{i}")
                      for i in range(2)]
                for g in range(2):
                    nc.sync.dma_start(
                        out=wp[g],
                        in_=wproj[l * D + g * 4 * P:l * D + (g + 1) * 4 * P, :]
                        .rearrange("(k p) c -> p k c", k=4))

                # ---- unpack gathered x_ln ----
                x_ag = ktp.tile([P, KT, SB], BD, tag="xag")
                for s in range(2):
                    nc.sync.dma_start(
                        out=x_ag[:, :, s * TOK:(s + 1) * TOK],
                        in_=agout[l][s * P:(s + 1) * P, :]
                        .rearrange("p (k t) -> p k t", k=KT))

                # ---- k for the whole batch: out [1024, SB] ----
                kTf = konp.tile([P, KT, SB], BD, tag="ktf")
                for m in range(KT):
                    pk = pmm.tile([P, SB], F32, tag="pmm")
                    for kk in range(KT):
                        nc.tensor.matmul(
                            pk,
                            wq[kk][:, D + m * P:D + (m + 1) * P],
                            x_ag[:, kk, :],
                            start=(kk == 0), stop=(kk == KT - 1))
                    nc.scalar.activation(kTf[:, m, :], pk,
                                         AF.Identity,
                                         bias=lb[:, 8 + m:9 + m])
                # ---- v (token-major into vf, ones col preserved) ----
                for tp in range(4):
                    for nn in range(2):
                        pv = pmm.tile([P, SB], F32, tag="pmm")
                        for kk in range(KT):
                            nc.tensor.matmul(
                                pv,
                                x_ag[:, kk, tp * P:(tp + 1) * P],
                                wq[kk][:, 2 * D + nn * 512:
                                       2 * D + (nn + 1) * 512],
                                start=(kk == 0), stop=False)
                        nc.tensor.matmul(
                            pv, ones_row,
                            vrow[:, nn * 512:(nn + 1) * 512],
                            start=False, stop=True)
                        nc.scalar.copy(
                            vf[:, tp, :].rearrange(
                                "p (g x) -> p g x", x=65)[:, nn * 8:(nn + 1) * 8, 0:64],
                            pv.rearrange("p (g x) -> p g x", x=64))

                # ---- attention: 8 head pairs, causal-superset chunks ----
                o_un = ounp.tile([P, KT, TOK], BD)

                for j in range(KT):
                    hA, hB = 2 * j, 2 * j + 1
                    # scores: separate PSUM tile per (cc, half)
                    sc = {}
                    for cc in (0, 2, 1, 3):
                        w = TOK if cc % 2 == 0 else P
                        for half, po in ((0, 0), (1, 64)):
                            t = pq4.tile([P, w], F32, tag="q4",
                                         name=f"sc{cc}_{half}_{l}_{j}")
                            sc[(cc, half)] = t
                            nc.tensor.matmul(
                                t,
                                kTf[po:po + 64, j,
                                    BBI[cc] * P:(BBI[cc] + 1) * P],
                                qT[po:po + 64, j,
                                   (0 if cc % 2 == 0 else P):TOK],
                                start=True, stop=True)
                    # scaled scores + additive mask (frees the PSUM
                    # bank fast), then exp from SBUF; er layout [A|B]
                    er = {}
                    for cc in (0, 2, 1, 3):
                        w = TOK if cc % 2 == 0 else P
                        scb = scbp.tile([P, 2 * w], BD, tag="scb",
                                        name=f"scb{cc}_{l}_{j}")
                        for half in (0, 1):
                            nc.vector.scalar_tensor_tensor(
                                out=scb[:, half * w:(half + 1) * w],
                                in0=sc[(cc, half)],
                                scalar=scale_col,
                                in1=mask_sb[:, cc,
                                            half * w:(half + 1) * w],
                                op0=OP.mult, op1=OP.add)
                        er[cc] = erp.tile([P, 2 * w], BD, tag="er",
                                          name=f"er{cc}_{l}_{j}")
                        nc.scalar.activation(er[cc], scb, AF.Exp)
                    # av: accumulate over chunks into [65, TOK] per head
                    for half, hh in ((0, hA), (1, hB)):
                        pav = pavp.tile([65, TOK], F32, tag="pav")
                        nc.tensor.matmul(
                            pav, vf[:, 0, hh * 65:(hh + 1) * 65],
                            er[0][:, half * TOK:(half + 1) * TOK],
                            start=True, stop=False)
                        nc.tensor.matmul(
                            pav, vf[:, 2, hh * 65:(hh + 1) * 65],
                            er[2][:, half * TOK:(half + 1) * TOK],
                            start=False, stop=False)
                        nc.tensor.matmul(
                            pav[:, P:TOK], vf[:, 1, hh * 65:(hh + 1) * 65],
                            er[1][:, half * P:(half + 1) * P],
                            start=False, stop=False)
                        nc.tensor.matmul(
                            pav[:, P:TOK], vf[:, 3, hh * 65:(hh + 1) * 65],
                            er[3][:, half * P:(half + 1) * P],
                            start=False, stop=True)
                        dsb = rfp.tile([1, TOK], F32, tag="dsb",
                                       name=f"dsb{l}_{j}_{half}")
                        nc.scalar.copy(dsb, pav[64:65, :])
                        o_tmp = invp.tile([64, TOK], BD, tag="otmp",
                                          name=f"otmp{l}_{j}_{half}")
                        nc.vector.tensor_copy(out=o_tmp, in_=pav[0:64, :])
                        rf_h = rfp.tile([1, TOK], F32, tag="rf",
                                        name=f"rf{l}_{j}_{half}")
                        nc.vector.reciprocal_approx_fast(
                            out=rf_h, in_=dsb)
                        rb_h = rfp.tile([1, TOK], BD, tag="rfb",
                                        name=f"rb{l}_{j}_{half}")
                        nc.vector.tensor_copy(out=rb_h, in_=rf_h)
                        pB = pavp.tile([64, TOK], F32, tag="pav",
                                       name=f"pB{l}_{j}_{half}")
                        nc.tensor.matmul(pB, ones_row[:, 0:64], rb_h,
                                         start=True, stop=True)
                        isb = invp.tile([64, TOK], BD, tag="isb",
                                        name=f"isb{l}_{j}_{half}")
                        nc.scalar.copy(isb, pB)
                        nc.vector.tensor_mul(
                            o_un[half * 64:(half + 1) * 64, j, :],
                            o_tmp, isb)

                wf = [wpool.tile([P, FF], BD, tag="w", name=f"wf{l}_{i}")
                      for i in range(KT)]
                for kk in range(KT):
                    nc.sync.dma_start(
                        out=wf[kk],
                        in_=wfc[l * D + kk * P:l * D + (kk + 1) * P, :])

                # ---- proj + residual: h_b = h_a + proj(o) + bias ----
                for m in range(KT):
                    pp = pmm.tile([P, SB], F32, tag="pmm")
                    for kk in range(KT):
                        nc.tensor.matmul(
                            pp[:, :TOK],
                            wp[kk // 4][:, kk % 4, m * P:(m + 1) * P],
                            o_un[:, kk, :],
                            start=(kk == 0), stop=(kk == KT - 1))
                    nc.vector.scalar_tensor_tensor(
                        out=h_b[:, m, :],
                        in0=pp[:, :TOK],
                        scalar=lb[:, 24 + m:25 + m],
                        in1=h_a[:, m, :],
                        op0=OP.add, op1=OP.add)

                # ---- LN2 + FC + gelu ----
                x_ln2 = xlnp.tile([P, KT, TOK], BD, tag="xln")
                emit_ln(h_b, x_ln2)
                g_sb = gsb.tile([P, FFT, TOK], BD)
                for m in range(FFT):
                    pf = pmm.tile([P, SB], F32, tag="pmm")
                    for kk in range(KT):
                        nc.tensor.matmul(
                            pf[:, :TOK],
                            wf[kk][:, m * P:(m + 1) * P],
                            x_ln2[:, kk, :],
                            start=(kk == 0), stop=(kk == KT - 1))
                    nc.scalar.activation(g_sb[:, m, :], pf[:, :TOK],
                                         AF.Gelu_apprx_tanh,
                                         bias=lb[:, 32 + m:33 + m])
                # ---- MLP (kk-outer, streamed weights) + residual ----
                wm = [wpool.tile([P, 4, D], BD, tag="w", name=f"wm{l}_{i}")
                      for i in range(8)]
                for g in range(8):
                    nc.sync.dma_start(
                        out=wm[g],
                        in_=wmlp[l * FF + g * 4 * P:l * FF + (g + 1) * 4 * P, :]
                        .rearrange("(k p) c -> p k c", k=4))
                for ph in range(2):
                    pml = [pq4.tile([P, TOK], F32, tag="q4",
                                    name=f"pml{l}_{ph}_{i}")
                           for i in range(4)]
                    for kk in range(FFT):
                        for mb in range(4):
                            m = 4 * ph + mb
                            nc.tensor.matmul(
                                pml[mb],
                                wm[kk // 4][:, kk % 4, m * P:(m + 1) * P],
                                g_sb[:, kk, :],
                                start=(kk == 0), stop=(kk == FFT - 1))
                    for mb in range(4):
                        m = 4 * ph + mb
                        nc.vector.scalar_tensor_tensor(
                            out=h_a[:, m, :],
                            in0=pml[mb],
                            scalar=lb[:, 64 + m:65 + m],
                            in1=h_b[:, m, :],
                            op0=OP.add, op1=OP.add)

            # ---- final LN + lm_head ----
            x_lnf = xlnp.tile([P, KT, TOK], BD, tag="xln")
            emit_ln(h_a, x_lnf)
            for nn in range(NV):
                wid = 512 if nn < NV - 1 else V - (NV - 1) * 512
                wh = wpool.tile([P, KT, wid], BD, tag="w",
                                name=f"wh{nn}")
                nc.sync.dma_start(
                    out=wh,
                    in_=whead[nn * KT * P:(nn + 1) * KT * P, 0:wid]
                    .rearrange("(k p) c -> p k c", k=KT))
                for tp in range(2):
                    ph = pmm.tile([P, SB], F32, tag="pmm")
                    for kk in range(KT):
                        nc.tensor.matmul(
                            ph[:, 0:wid],
                            x_lnf[:, kk, tp * P:(tp + 1) * P],
                            wh[:, kk, :],
                            start=(kk == 0), stop=(kk == KT - 1))
                    ob = obp.tile([P, 512], BD)
                    nc.scalar.copy(ob[:, 0:wid], ph[:, 0:wid])
                    nc.sync.dma_start(
                        out=out[tp * P:(tp + 1) * P,
                                nn * 512:nn * 512 + wid],
                        in_=ob[:, 0:wid])

    nc.compile()
    return nc


# chunk cc -> column block (x128) inside kTf's gathered token axis.
# Gathered order IS rank order == BB order, so chunk cc sits at block cc.
BBI = [0, 1, 2, 3]


_CACHE = {}


def _get_nc(n_layers):
    if n_layers not in _CACHE:
        _CACHE[n_layers] = _build(n_layers)
    return _CACHE[n_layers]


def _prep_host(inputs, n_layers):
    """Host-side: embeddings, LN-affine folding, layouts, per-core shards."""
    ids = np.asarray(inputs["input_ids"])
    tts = np.asarray(inputs["token_type_ids"])
    wte = np.asarray(inputs["wte"], np.float32)
    wtte = np.asarray(inputs["wtte"], np.float32)
    wpe = np.asarray(inputs["wpe"], np.float32)

    h0 = wte[ids] + wpe[None, :, :] + wtte[tts]          # [B, S, D]

    ln1_w = np.asarray(inputs["ln1_w"], np.float32)
    ln1_b = np.asarray(inputs["ln1_b"], np.float32)
    attn_w = np.asarray(inputs["attn_w"], np.float32)
    attn_b = np.asarray(inputs["attn_b"], np.float32)
    atp_w = np.asarray(inputs["atp_w"], np.float32)
    atp_b = np.asarray(inputs["atp_b"], np.float32)
    ln2_w = np.asarray(inputs["ln2_w"], np.float32)
    ln2_b = np.asarray(inputs["ln2_b"], np.float32)
    fc_w = np.asarray(inputs["fc_w"], np.float32)
    fc_b = np.asarray(inputs["fc_b"], np.float32)
    mlp_w = np.asarray(inputs["mlp_w"], np.float32)
    mlp_b = np.asarray(inputs["mlp_b"], np.float32)
    lnf_w = np.asarray(inputs["lnf_w"], np.float32)
    lnf_b = np.asarray(inputs["lnf_b"], np.float32)
    head_w = np.asarray(inputs["head_w"], np.float32)
    head_b = np.asarray(inputs["head_b"], np.float32)

    nl = n_layers
    wqkv = np.empty((nl * D, 3 * D), BF)
    wproj_ = np.empty((nl * D, D), BF)
    wfc_ = np.empty((nl * D, FF), BF)
    wmlp_ = np.empty((nl * FF, D), BF)
    bvec = np.zeros((nl * P, 72), np.float32)
    bvrow = np.zeros((nl, D), BF)
    for l in range(nl):
        wq = attn_w[l] * ln1_w[l][:, None]
        bq = attn_b[l] + ln1_b[l] @ attn_w[l]            # [3072]
        wqkv[l * D:(l + 1) * D] = wq.astype(BF)
        wproj_[l * D:(l + 1) * D] = atp_w[l].astype(BF)
        wfc_[l * D:(l + 1) * D] = (fc_w[l] * ln2_w[l][:, None]).astype(BF)
        wmlp_[l * FF:(l + 1) * FF] = mlp_w[l].astype(BF)
        bvec[l * P:(l + 1) * P, 0:8] = bq[0:D].reshape(8, P).T
        bvec[l * P:(l + 1) * P, 8:16] = bq[D:2 * D].reshape(8, P).T
        bvec[l * P:(l + 1) * P, 24:32] = atp_b[l].reshape(8, P).T
        bfc = fc_b[l] + ln2_b[l] @ fc_w[l]
        bvec[l * P:(l + 1) * P, 32:64] = bfc.reshape(32, P).T
        bvec[l * P:(l + 1) * P, 64:72] = mlp_b[l].reshape(8, P).T
        bvrow[l] = bq[2 * D:3 * D].astype(BF)            # v bias as row

    whf = (head_w * lnf_w[:, None]).astype(np.float32)
    whp = np.zeros((D, VPAD), np.float32)
    whp[:, :V] = whf
    whead = np.ascontiguousarray(
        whp.reshape(KT, P, NV, 512).transpose(2, 0, 1, 3)
    ).reshape(NV * KT * P, 512).astype(BF)
    bhost = lnf_b @ head_w + head_b                      # [V]


    in_maps = []
    for c in range(8):
        rho = c % 2
        batch = c // 2
        qb = QBLOCKS[rho]
        h0T = np.ascontiguousarray(
            np.concatenate(
                [h0[batch, qb[0] * P:(qb[0] + 1) * P],
                 h0[batch, qb[1] * P:(qb[1] + 1) * P]], axis=0).T
        ).astype(np.float32)                              # [D, TOK]
        # masks [P, 4*SB]: even cc [m|m], odd cc [m_q1|m_q1|zeros]
        mk = np.zeros((P, 4 * SB), BF)
        for cc in range(4):
            kb = BB[cc]
            m = np.zeros((P, TOK), np.float32)
            for qh in range(2):
                qblk = qb[qh]
                kg = kb * P + np.arange(P)[:, None]
                qg = qblk * P + np.arange(P)[None, :]
                m[:, qh * P:(qh + 1) * P] = \
                    np.where(kg <= qg, 0.0, -10000.0)
            if cc % 2 == 0:
                mk[:, cc * SB:cc * SB + TOK] = m.astype(BF)
                mk[:, cc * SB + TOK:(cc + 1) * SB] = m.astype(BF)
            else:
                mq1 = m[:, P:TOK]
                mk[:, cc * SB:cc * SB + P] = mq1.astype(BF)
                mk[:, cc * SB + P:cc * SB + TOK] = mq1.astype(BF)
        in_maps.append({
            "h0T": h0T,
            "wqkv": wqkv, "wproj": wproj_, "wfc": wfc_, "wmlp": wmlp_,
            "whead": whead, "bvec": bvec, "bvrow": bvrow,
            "masks": mk,
        })
    return in_maps, bhost


def kernel(**inputs):
    from concourse import bass_utils

    n_layers = N_LAYERS
    nc = _get_nc(n_layers)
    in_maps, bhost = _prep_host(inputs, n_layers)

    trace = bool(int(os.environ.get("GPT2_TRACE", "0")))
    res = bass_utils.run_bass_kernel_spmd(
        nc, in_maps, core_ids=list(range(8)), trace=trace)
    if trace:
        kernel.last_exec_time_ns = res.exec_time_ns
        kernel.last_results = res

    full = np.empty((B, S, V), np.float32)
    for c in range(8):
        o = res.results[c]["out"]                         # [TOK, VPAD]
        rho = c % 2
        batch = c // 2
        qb = QBLOCKS[rho]
        full[batch, qb[0] * P:(qb[0] + 1) * P] = o[0:P, :V]
        full[batch, qb[1] * P:(qb[1] + 1) * P] = o[P:2 * P, :V]
    full += bhost[None, None, :]
    return full
